# revision 1
# baseline (speedup 1.0000x reference)
"""Directional contrastive loss on 8 Trainium2 NeuronCores.

Math: with all labels equal (per the problem spec) the mask is all-ones and

  loss = mean_{n,i,j} log(denom + 1e-6)        ... (over N*H*W)
         - mean_{n,m,i,j} logits               ... (over N*M*H*W)

  logits[n,m,i,j] = <fn[n,:,i,j], fn[n,:, i+d0[m,i,j], j+d1[m,i,j]]> / T
  denom[n,i,j]    = sum_m exp(logits[n,m,i,j])

Since (d0,d1) in {-1,0,1}^2, logits take at most 9 values per (n,i,j):
S_k[n,i,j] = cos(x[n,:,i,j], x[n,:,i+di,j+dj]) / T for the 9 offsets k.
With cnt_k[i,j] = #{m : dir_m(i,j) == k} (host-precomputed from the int32
`directions` tensor):

  sum_m logits       = sum_k cnt_k * S_k
  denom              = sum_k cnt_k * exp(S_k);  the self term k=(0,0) is
                       exactly exp(1/T) (cos = 1), folded into a host constant.

Host packing normalizes the features (y = x/||x|| * sqrt(1/T), bf16), so the
device computes S_d maps directly as channel reductions of shifted products
of y: 4 shifted maps (the other 4 follow by symmetry S_{-k}[p] = S_k[p - k]).

Sharding: by batch — core n owns batch n (the cross-batch coupling lives
entirely in the tiny replicated cnt maps, so no halos are needed).

Device pipeline per core:
  - products y * shift_d(y) on DVE in bf16.  DVE's 2x mode requires 4-byte
    alignment on every operand, and three of the four shifts are ODD bf16
    element offsets (+1, +111, +113) — so the scalar engine materializes a
    one-element-shifted copy (x1o/x2o) of each feature tile, making every
    product read even-offset (4B-aligned) and keeping all products at 2x.
  - channel reduction on the tensor engine as selector-column matmuls
    accumulating into psum partition rows (4 map quadrants of a [128, 448]
    psum tile; the 64-channel overhang rides a second tensor packed as two
    pixel halves, folded in via a two-column selector).
  - softmax-style assembly (exp / cnt-weighted sums / log) on scalar+DVE.
  - per-rep stage 2 is emitted AFTER the next rep's products so the DVE
    FIFO never head-of-line blocks on the scalar exp / scatter DMAs, and
    input tiles double-buffer so rep i+1's DMAs land under rep i's compute.

Each core returns per-partition (= per image row) partial sums in a
[128, 3] tensor; the host adds them up and scales.
"""

import os
import sys

import numpy as np

for _p in ("/opt/trn_rl_repo", "/root/.axon_site/_ro/trn_rl_repo"):
    if os.path.isdir(_p) and _p not in sys.path:
        sys.path.insert(0, _p)

import contextlib

import concourse.bacc as bacc
import concourse.mybir as mybir
from concourse import tile
from concourse.bass_utils import run_bass_kernel_spmd

from ml_dtypes import bfloat16

N, C, H, W = 8, 192, 112, 112
TEMP = 0.1
CORES = 8                # core n owns batch n (no spatial halos needed)
PIX = H * W              # 12544 pixels per core
X2R = 57                 # rows per x2 half (56 owned + 1 partner row)
X2W = X2R * W            # 6384
PAD = 128                # column padding on the packed feature tiles
CH = 448                 # psum chunk (4 partition-rows x 112)
NQ = PIX // CH           # 28 chunks per map
BLK = 6272               # x1 product block = 14 chunks
NBLK = PIX // BLK        # 2
QPB = BLK // CH          # 14
DMACH = 3136             # x1 input DMA chunk (finer than product blocks)
GCH = 3136               # odd-copy generation chunk
X2SPLIT = 3136           # x2 input DMA split
X2P = 6272               # x2 product width (cols consumed by the matmuls)
NCH_CM = 10              # constant-map channels
X1L = PAD + PIX + PAD    # 12800
X2L = PAD + X2W + PAD    # 6640

_dt = mybir.dt
_F32 = _dt.float32
_BF16 = _dt.bfloat16

# shift offsets in pixel-linear space for maps m=1..4:
# 1: (0,+1), 2: (+1,-1), 3: (+1,0), 4: (+1,+1)
DELTAS = {1: 1, 2: W - 1, 3: W, 4: W + 1}
MAPS = (3, 1, 2, 4)      # even-offset map first: its products need no odd copy


def _cap(base, dims, off):
    """Custom access pattern: keep base's partition dim, replace the free
    dims with `dims` ([stride, count] outer->inner) at element offset `off`."""
    import bass_rust
    return bass_rust.AP(tensor=base.tensor, offset=base.offset + off,
                        ap=[list(base.ap[0])] + [list(d) for d in dims])


def emit_consts(nc, cp, cmd):
    """Constants + one-time setup shared by every rep."""
    AF = mybir.ActivationFunctionType
    cmt = cp.tile([128, NCH_CM * W], _F32, tag="cmt", name="cmt")
    # Stationary selector banks: z_ones[*, 31-q:63-q] puts the selector
    # column at position q of a [128, 32] lhsT, so an M=32 matmul accumulates
    # one result row into psum row q while adding 0 to the other rows.
    z_ones = cp.tile([128, 63], _BF16, tag="z_ones", name="z_ones")
    # z2 carries TWO selector columns 14 apart for the x2 pixel-half fold.
    z2 = cp.tile([128, 63], _BF16, tag="z2", name="z2")
    S4 = cp.tile([128, 5, 114], _F32, tag="S4", name="S4")
    DvDs = cp.tile([128, W], _F32, tag="DvDs", name="DvDs")
    biaseps = cp.tile([128, 1], _F32, tag="biaseps", name="biaseps")
    warm = cp.tile([128, 1], _F32, tag="warm", name="warm")

    for z in (z_ones, z2):
        nc.gpsimd.memset(z[:], 0.0)
    nc.gpsimd.memset(z_ones[:, 31:32], 1.0)
    nc.gpsimd.memset(z2[0:64, 31:32], 1.0)
    nc.gpsimd.memset(z2[64:128, 45:46], 1.0)
    # S4 pads (cols 0/113, rows 112+) stay 0 forever; the scatter DMAs only
    # ever write [0:112, :, 1:113].  exp(0)=1 at pads is masked by cnt=0.
    nc.gpsimd.memset(S4[:], 0.0)
    nc.gpsimd.memset(DvDs[0:1, :], 0.0)
    nc.gpsimd.memset(biaseps[:], 1e-6)
    nc.gpsimd.memset(warm[:], 1.0)
    for fn in (AF.Exp, AF.Ln):
        nc.scalar.activation(out=warm[:], in_=warm[:], func=fn)

    nc.sync.dma_start(out=cmt[:], in_=cmd[:])
    return dict(cmt=cmt, z_ones=z_ones, z2=z2, S4=S4, DvDs=DvDs,
                biaseps=biaseps)


def emit_rep(nc, pools, cst, x1d, x2d, outd):
    """Emit one rep's input DMAs, odd copies, products and matmuls.

    Returns a closure that emits the rep's stage 2 (psum evacuation, exp,
    cnt-weighted sums, log) — called AFTER the next rep's front has been
    emitted so stage 2 lands behind the next rep's products in the DVE/scalar
    FIFOs instead of head-of-line blocking them.
    """
    AF = mybir.ActivationFunctionType
    OP = mybir.AluOpType
    xp, pp, sp, qp, obp = pools
    cmt, z_ones, z2, S4 = cst["cmt"], cst["z_ones"], cst["z2"], cst["S4"]
    DvDs, biaseps = cst["DvDs"], cst["biaseps"]

    x1e = xp.tile([128, X1L], _BF16, tag="x1e", name="x1e", bufs=2)
    x2e = xp.tile([128, X2L], _BF16, tag="x2e", name="x2e", bufs=2)
    # odd copies: x1o[k] = x1e[k+1]; single-buffered (range-level deps let
    # the next rep's generation start as soon as this rep's early blocks
    # have been consumed).
    x1o = xp.tile([128, X1L], _BF16, tag="x1o", name="x1o", bufs=1)
    x2o = xp.tile([128, X2L], _BF16, tag="x2o", name="x2o", bufs=1)

    # ---- input DMAs (chunked so consumers unlock progressively) ----
    nc.sync.dma_start(out=x1e[:, 0:2 * PAD], in_=x1d[:, 0:2 * PAD])
    nc.sync.dma_start(out=x1e[:, 2 * PAD:2 * PAD + DMACH],
                      in_=x1d[:, 2 * PAD:2 * PAD + DMACH])
    h1 = PAD + X2SPLIT + PAD
    nc.sync.dma_start(out=x2e[:, 0:h1], in_=x2d[:, 0:h1])
    nc.sync.dma_start(out=x2e[:, h1:X2L], in_=x2d[:, h1:X2L])
    for b in range(1, PIX // DMACH):
        c0 = 2 * PAD + b * DMACH
        nc.sync.dma_start(out=x1e[:, c0:c0 + DMACH], in_=x1d[:, c0:c0 + DMACH])

    # ---- odd copies on the (otherwise idle) scalar engine ----
    def gen_odd(dst, src, gs, ge):
        nc.scalar.activation(out=dst[:, gs:ge], in_=src[:, gs + 1:ge + 1],
                             func=AF.Copy)

    gen_odd(x1o, x1e, PAD, PAD + GCH + 127)
    gen_odd(x2o, x2e, PAD, PAD + X2SPLIT + 127)
    gen_odd(x2o, x2e, PAD + X2SPLIT, PAD + X2W + PAD // 2)
    for b in range(1, PIX // GCH):
        gs = PAD + b * GCH
        gen_odd(x1o, x1e, gs, gs + GCH + 127)

    # ---- stage 1: products (DVE, all at 2x) + selector matmuls ----
    ptA = qp.tile([128, CH], _F32, tag="psA", name="psA")

    def quad(m):
        return ptA[(m - 1) * 32:m * 32, :], (0, (m - 1) * 32)

    prod2 = {}
    for b in range(NBLK):
        s = PAD + b * BLK
        prods = {}
        for m in MAPS:
            t = pp.tile([128, BLK], _BF16, tag="prod", name="prod", bufs=3)
            d = DELTAS[m]
            if d % 2 == 0:
                in1 = x1e[:, s + d:s + d + BLK]
            else:
                in1 = x1o[:, s + d - 1:s + d - 1 + BLK]
            nc.vector.tensor_tensor(out=t[:], in0=x1e[:, s:s + BLK],
                                    in1=in1, op=OP.mult)
            prods[m] = t
        for m in MAPS:
            dst, tpos = quad(m)
            for c in range(QPB):
                q = b * QPB + c
                nc.tensor.matmul(dst, z_ones[:, 31 - q:63 - q],
                                 prods[m][:, c * CH:(c + 1) * CH],
                                 start=(q == 0), stop=(q == NQ - 1),
                                 tile_position=tpos, skip_group_check=True)
        if b == 0:
            for m in MAPS:
                t2 = pp.tile([128, X2P], _BF16, tag="prod2", name="prod2",
                             bufs=2)
                d = DELTAS[m]
                if d % 2 == 0:
                    in1 = x2e[:, PAD + d:PAD + d + X2P]
                else:
                    in1 = x2o[:, PAD + d - 1:PAD + d - 1 + X2P]
                nc.vector.tensor_tensor(out=t2[:],
                                        in0=x2e[:, PAD:PAD + X2P],
                                        in1=in1, op=OP.mult)
                prod2[m] = t2
            for m in MAPS:
                dst, tpos = quad(m)
                for r in range(14):
                    nc.tensor.matmul(dst, z2[:, 31 - r:63 - r],
                                     prod2[m][:, r * CH:(r + 1) * CH],
                                     start=False, stop=False,
                                     tile_position=tpos,
                                     skip_group_check=True)

    def finish():
        # ---- psum evacuation + scatter into the stacked S4 logit maps ----
        stA = sp.tile([128, CH], _F32, tag="stA", name="stA", bufs=2)
        for m in MAPS:
            r0 = (m - 1) * 32
            nc.scalar.activation(out=stA[r0:r0 + NQ, :],
                                 in_=ptA[r0:r0 + NQ, :], func=AF.Copy)
        for m in MAPS:
            nc.sync.dma_start(out=S4[0:112, m - 1, 1:113],
                              in_=stA[(m - 1) * 32:(m - 1) * 32 + NQ, :])
        # slot 4 = map 1 again, one column later: entry k=4 of the stacked
        # APs then reads S1[p, j-1] — the single (0,-1) derived term — so it
        # folds into the same 5-entry multiply/reduce as the direct maps.
        nc.sync.dma_start(out=S4[0:112, 4, 2:114], in_=stA[0:NQ, :])

        # ---- stage 2: softmax-style assembly ----
        E4 = sp.tile([128, 5, 114], _F32, tag="E4", name="E4", bufs=1)
        nc.scalar.activation(out=E4[:], in_=S4[:], func=AF.Exp)

        outsb = obp.tile([128, 3], _F32, tag="outsb", name="outsb", bufs=2)
        # contribution source APs, element order (j, stacked-entry):
        #  direct+single: maps 1..4 at col j+1 and map 1 (slot 4) at col j,
        #          cnt channels 2..6 (A, B, C, D, Am)
        #  derived (at source partition): maps 2,3,4 at cols j+2,j+1,j,
        #          cnt channels 7,8,9 (host-pre-shifted one row up)
        cdir = _cap(cmt[:], [[1, W], [W, 5]], 2 * W)
        edir = _cap(E4[:], [[1, W], [114, 5]], 1)
        sdir = _cap(S4[:], [[1, W], [114, 5]], 1)
        cder = _cap(cmt[:], [[1, W], [W, 3]], 7 * W)
        eder = _cap(E4[:], [[1, W], [113, 3]], 116)
        sder = _cap(S4[:], [[1, W], [113, 3]], 116)

        # denominator side
        tdD = sp.tile([128, W, 5], _F32, tag="tdD", name="tdD", bufs=1)
        nc.vector.tensor_tensor(out=tdD[:], in0=cdir, in1=edir, op=OP.mult)
        sumD = sp.tile([128, W], _F32, tag="sumD", name="sumD", bufs=1)
        nc.vector.reduce_sum(out=sumD[:], in_=tdD[:],
                             axis=mybir.AxisListType.X)
        Dv = sp.tile([128, W], _F32, tag="Dv", name="Dv", bufs=1)
        nc.vector.tensor_tensor(out=Dv[:], in0=cmt[:, 0:W], in1=sumD[:],
                                op=OP.add)
        tvD = sp.tile([128, W, 3], _F32, tag="tvD", name="tvD", bufs=1)
        nc.vector.tensor_tensor(out=tvD[:], in0=cder, in1=eder, op=OP.mult)
        DvD = sp.tile([128, W], _F32, tag="DvD", name="DvD", bufs=1)
        nc.vector.reduce_sum(out=DvD[:], in_=tvD[:],
                             axis=mybir.AxisListType.X)
        # derived contributions belong one partition DOWN; realize with DMA
        nc.sync.dma_start(out=DvDs[1:128, :], in_=DvD[0:127, :])

        # logit-sum side (fills the DVE while the DvDs DMA flies); the full
        # reduces ride the scalar engine's accum_out to keep DVE cycles low.
        tdS = sp.tile([128, W, 5], _F32, tag="tdS", name="tdS", bufs=1)
        nc.vector.tensor_tensor(out=tdS[:], in0=cdir, in1=sdir, op=OP.mult)
        scD4 = sp.tile([128, W, 5], _F32, tag="scD4", name="scD4", bufs=1)
        nc.scalar.activation(out=scD4[:], in_=tdS[:], func=AF.Copy,
                             accum_out=outsb[:, 1:2])
        tvS = sp.tile([128, W, 3], _F32, tag="tvS", name="tvS", bufs=1)
        nc.vector.tensor_tensor(out=tvS[:], in0=cder, in1=sder, op=OP.mult)
        scD3 = sp.tile([128, W, 3], _F32, tag="scD3", name="scD3", bufs=1)
        nc.scalar.activation(out=scD3[:], in_=tvS[:], func=AF.Copy,
                             accum_out=outsb[:, 2:3])

        nc.vector.tensor_tensor(out=Dv[:], in0=Dv[:], in1=DvDs[:], op=OP.add)
        lgd = sp.tile([128, W], _F32, tag="lgd", name="lgd", bufs=1)
        nc.scalar.activation(out=lgd[:], in_=Dv[:], func=AF.Ln,
                             bias=biaseps[:], accum_out=outsb[:, 0:1])
        nc.sync.dma_start(out=outd[:], in_=outsb[:])

    return finish


def build_nc(reps=1):
    """Build + compile the SPMD program. reps>1 unrolls the whole body for
    device-side timing (amortizes the axon tunnel round-trip)."""
    nc = bacc.Bacc("TRN2", target_bir_lowering=False, debug=False,
                   num_devices=CORES)
    x1d = nc.dram_tensor("x1", [128, X1L], _BF16, kind="ExternalInput")
    x2d = nc.dram_tensor("x2", [128, X2L], _BF16, kind="ExternalInput")
    cmd = nc.dram_tensor("cm", [128, NCH_CM * W], _F32, kind="ExternalInput")
    outd = nc.dram_tensor("out", [128, 3], _F32, kind="ExternalOutput")
    with contextlib.ExitStack() as ctx:
        with tile.TileContext(nc) as tc:
            with tc.tile_pool(name="constp", bufs=1) as cp, \
                 tc.tile_pool(name="xp", bufs=2) as xp, \
                 tc.tile_pool(name="prodp", bufs=4) as pp, \
                 tc.tile_pool(name="s2p", bufs=1) as sp, \
                 tc.tile_pool(name="outp", bufs=2) as obp, \
                 tc.tile_pool(name="psump", bufs=2, space="PSUM") as qp:
                cst = emit_consts(nc, cp, cmd)
                pools = (xp, pp, sp, qp, obp)
                fin = None
                for _ in range(reps):
                    nxt = emit_rep(nc, pools, cst, x1d, x2d, outd)
                    if fin is not None:
                        fin()
                    fin = nxt
                fin()
    nc.compile()
    return nc


def pack_inputs(features, labels, directions):
    """Host-side sharding/packing. Returns per-core input dicts."""
    features = np.asarray(features, dtype=np.float32)
    directions = np.asarray(directions)
    labels = np.asarray(labels)

    # normalize + fold the 1/T temperature split across both dot factors:
    # y = x / ||x|| * sqrt(1/T)  =>  <y, y'> = cos(x, x') / T
    nrm = np.sqrt((features * features).sum(axis=1, keepdims=True))
    y = features / np.maximum(nrm, np.float32(1e-12))
    y *= np.float32(np.sqrt(1.0 / TEMP))

    # direction histogram over the batch axis: cnt[k, i, j]
    k = (directions[:, 0].astype(np.int64) + 1) * 3 + (directions[:, 1] + 1)
    cnt = np.zeros((9, H, W), np.float32)
    for kk in range(9):
        cnt[kk] = (k == kk).sum(axis=0)

    # If labels are not all identical, fall back to a mask-aware host path
    # (the problem spec fills labels with zeros, so this never triggers).
    uniform_labels = (labels == labels.flat[0]).all()

    e_self = np.exp(np.float32(1.0 / TEMP))
    # constant-map channels shared by all cores (partition p = image row i)
    # channel order: den0, lg0, A, B, C, D, Am, Bm, Cm, Dm — the four direct
    # counts plus Am sit adjacent so the device reads them as one 5-entry
    # stride-W AP; Bm/Cm/Dm are the derived-at-source counts.
    # (lg0 is no longer read on-device: the self logit-sum term is constant
    # and is added host-side in unpack_loss)
    ch = np.empty((NCH_CM, H, W), np.float32)
    ch[0] = cnt[4] * e_self
    ch[1] = cnt[4] * np.float32(1.0 / TEMP)
    global _SELF_LS
    _SELF_LS = float(np.float64(ch[1]).sum())
    for i, kk in enumerate((5, 6, 7, 8, 3, 2, 1, 0)):
        ch[2 + i] = cnt[kk]
    chT = ch.transpose(1, 0, 2)                          # (H, NCH, W)
    cm = np.zeros((128, NCH_CM, W), np.float32)
    plain = [0, 1, 2, 3, 4, 5, 6]
    shifted = [7, 8, 9]                                  # derived-at-source
    cm[0:H, plain] = chT[:, plain]
    cm[0:H - 1, shifted] = chT[1:H, shifted]
    cm = np.ascontiguousarray(cm.reshape(128, NCH_CM * W))

    in_maps = []
    for core in range(CORES):
        xb = y[core].astype(bfloat16)                    # (C, H, W)
        x1 = np.zeros((128, X1L), bfloat16)
        x1[:, PAD:PAD + PIX] = xb[:128].reshape(128, PIX)
        hi = xb[128:]                                    # (64, H, W)
        x2 = np.zeros((128, X2L), bfloat16)
        x2[0:64, PAD:PAD + X2W] = hi[:, 0:X2R].reshape(64, X2W)
        lower_rows = np.clip(np.arange(56, 56 + X2R), 0, H - 1)
        x2[64:128, PAD:PAD + X2W] = hi[:, lower_rows].reshape(64, X2W)
        in_maps.append({"x1": x1, "x2": x2, "cm": cm})
    return in_maps, uniform_labels


_SELF_LS = 0.0


def unpack_loss(results):
    """Combine per-core [128, 3] partials into the scalar loss.

    cols: 0 = sum_j log(denom), 1 = direct + single logit sums, 2 = derived
    logit sums (at source rows).  The self-map logit sum (cnt4/T, a host
    constant) is added here.
    """
    lg_sum = 0.0
    ls_sum = 0.0
    for core in range(CORES):
        o = np.asarray(results[core]["out"], np.float64)
        lg_sum += o[0:H, 0].sum()
        ls_sum += o[0:H, 1].sum() + o[0:H - 1, 2].sum() + _SELF_LS
    loss = lg_sum / (N * H * W) - ls_sum / (N * N * H * W)
    return np.float32(loss)


_NC_CACHE = {}


def _get_nc(reps=1):
    if reps not in _NC_CACHE:
        _NC_CACHE[reps] = build_nc(reps)
    return _NC_CACHE[reps]


def _host_reference_loss(features, labels, directions):
    """Mask-aware fallback (numpy, fp32) for non-uniform labels."""
    f = np.asarray(features, np.float32)
    nrm = np.sqrt((f * f).sum(axis=1, keepdims=True))
    fn = f / np.maximum(nrm, 1e-12)
    ii = np.arange(H)[None, :, None]
    jj = np.arange(W)[None, None, :]
    ni = ii + directions[:, 0]
    nj = jj + directions[:, 1]
    gathered = fn[:, :, ni, nj]                 # (N, C, M, H, W)
    logits = np.einsum('ncij,ncmij->nmij', fn, gathered) / TEMP
    lab = np.asarray(labels)
    labels_g = lab[:, ni, nj]
    mask = (lab[None, :, :, :] == labels_g).astype(np.float32)
    exp_l = np.exp(logits) * mask
    denom = exp_l.sum(axis=1, keepdims=True)
    return np.float32((-np.log(exp_l / (denom + 1e-6))).mean())


def kernel(features, labels, directions):
    in_maps, uniform = pack_inputs(features, labels, directions)
    if not uniform:
        return _host_reference_loss(features, labels, directions)
    nc = _get_nc()
    res = run_bass_kernel_spmd(nc, in_maps, core_ids=list(range(CORES)))
    return unpack_loss(res.results)



# revision 15
# speedup vs baseline: 1.4706x; 1.4706x over previous
"""Directional contrastive loss on 8 Trainium2 NeuronCores.

Math: with all labels equal (per the problem spec) the mask is all-ones and

  loss = mean_{n,i,j} log(denom + 1e-6)        ... (over N*H*W)
         - mean_{n,m,i,j} logits               ... (over N*M*H*W)

  logits[n,m,i,j] = <fn[n,:,i,j], fn[n,:, i+d0[m,i,j], j+d1[m,i,j]]> / T
  denom[n,i,j]    = sum_m exp(logits[n,m,i,j])

Since (d0,d1) in {-1,0,1}^2, logits take at most 9 values per (n,i,j):
S_k[n,i,j] = cos(x[n,:,i,j], x[n,:,i+di,j+dj]) / T for the 9 offsets k.
With cnt_k[i,j] = #{m : dir_m(i,j) == k} (host-precomputed from the int32
`directions` tensor):

  sum_m logits       = sum_k cnt_k * S_k
  denom              = sum_k cnt_k * exp(S_k);  the self term k=(0,0) is
                       exactly exp(1/T) (cos = 1), folded into a host constant.

Host packing normalizes the features (y = x/||x|| * sqrt(1/T), bf16), so the
device computes S_d maps directly as channel reductions of shifted products
of y: 4 shifted maps (the other 4 follow by symmetry S_{-k}[p] = S_k[p - k]).

Sharding: by batch — core n owns batch n (the cross-batch coupling lives
entirely in the tiny replicated cnt maps, so no halos are needed).

Device pipeline per core (v2 - "flipped" PE reduction):
  - products y * shift_d(y) on DVE in bf16 2x (odd-element shifts read a
    one-element-shifted scalar-engine copy so every operand is 4B-aligned),
    with a tunable slice of the product columns offloaded to GPSIMD (Pool).
  - channel reduction on the tensor engine with the PRODUCT CHUNK AS THE
    STATIONARY WEIGHTS: per image row a [128, 128] Ldweights (full-width
    bf16 weight loads ride the fast-weight-load path at 2 cols/cycle) and a
    1-2 column matmul against a constant selector column sums the 128
    channel partitions into psum partition j (pixel column), psum col i
    (row).  The 64-channel overhang tensor packs two row-halves on 128
    partitions; its two masked selector columns accumulate rows r and r+56
    into the same psum map region (start=False).  This halves the PE cost
    vs streaming the products as moving data.
  - psum therefore holds the four S maps TRANSPOSED ([j, i]); the softmax
    assembly works in that layout: one scalar copy evacuates all 4 maps
    into a 6-slot stacked tile (2 extra DMA-scattered slots give the
    column-shifted reads), exp on scalar, cnt-weighted sums on DVE with
    stacked APs, two partition-shift DMAs realize the derived-term
    row+-1 moves, Ln + accum_out per-partition partials.
  - per-rep stage 2 is emitted AFTER the next rep's products so the DVE
    FIFO never head-of-line blocks on the scalar exp / scatter DMAs.

Each core returns per-partition (= per image column j) partial sums in a
[128, 3] tensor; the host adds them up and scales.
"""

import os
import sys

import numpy as np

for _p in ("/opt/trn_rl_repo", "/root/.axon_site/_ro/trn_rl_repo"):
    if os.path.isdir(_p) and _p not in sys.path:
        sys.path.insert(0, _p)

import contextlib

import concourse.bacc as bacc
import concourse.mybir as mybir
from concourse import tile
from concourse.bass_utils import run_bass_kernel_spmd

from ml_dtypes import bfloat16

N, C, H, W = 8, 192, 112, 112
TEMP = 0.1
CORES = 8                # core n owns batch n (no spatial halos needed)
PIX = H * W              # 12544 pixels per core
X2R = 57                 # rows per x2 half (56 owned + 1 partner row)
X2W = X2R * W            # 6384
PAD = 128                # column padding on the packed feature tiles
BLK = 6272               # product block = 56 image rows
NBLK = PIX // BLK        # 2
PPAD = BLK + 16          # product tile width (16-col tail for ldweights pad)
DMACH = 3136             # x1 input DMA chunk
GCH = 3136               # odd-copy generation chunk
X2SPLIT = 3136           # x2 input DMA split
NCH_CM = 12              # constant-map channels
X1L = PAD + PIX + PAD + 16   # 12816 (tail so odd copies cover product pads)
X2L = PAD + X2W + PAD    # 6640
RPB = BLK // W           # 56 rows per block

_dt = mybir.dt
_F32 = _dt.float32
_BF16 = _dt.bfloat16

# shift offsets in pixel-linear space for maps m=1..4:
# 1: (0,+1), 2: (+1,-1), 3: (+1,0), 4: (+1,+1)
DELTAS = {1: 1, 2: W - 1, 3: W, 4: W + 1}
MAPS = (3, 1, 2, 4)      # even-offset map first: its products need no odd copy
CB = {m: (m - 1) * W for m in (1, 2, 3, 4)}   # psum col base per map

# product columns computed on GPSIMD instead of DVE, per (block, map).
# block 2 is the x2 overhang tensor.  Tune by measurement.
POOL_SPLIT = {}


def _cap(base, dims, off):
    """Custom access pattern: keep base's partition dim, replace the free
    dims with `dims` ([stride, count] outer->inner) at element offset `off`."""
    import bass_rust
    return bass_rust.AP(tensor=base.tensor, offset=base.offset + off,
                        ap=[list(base.ap[0])] + [list(d) for d in dims])


def emit_consts(nc, cp, cmd):
    """Constants + one-time setup shared by every rep."""
    AF = mybir.ActivationFunctionType
    cmt = cp.tile([128, NCH_CM * W], _F32, tag="cmt", name="cmt")
    # selector columns for the flipped reduction: col 0 = ones (x1 chunks),
    # col 1 = ones on partitions 0:64 (x2 half A), col 2 = ones on 64:128.
    selm = cp.tile([128, 3], _BF16, tag="selm", name="selm")
    S4T = cp.tile([128, 6, 114], _F32, tag="S4T", name="S4T")
    DvPs = cp.tile([128, W], _F32, tag="DvPs", name="DvPs")
    DvMs = cp.tile([128, W], _F32, tag="DvMs", name="DvMs")
    biaseps = cp.tile([128, 1], _F32, tag="biaseps", name="biaseps")
    warm = cp.tile([128, 1], _F32, tag="warm", name="warm")

    nc.gpsimd.memset(selm[:], 0.0)
    nc.gpsimd.memset(selm[:, 0:1], 1.0)
    nc.gpsimd.memset(selm[0:64, 1:2], 1.0)
    nc.gpsimd.memset(selm[64:128, 2:3], 1.0)
    # S4T pads stay 0 forever; the evac/scatter writes only the interior.
    # exp(0)=1 at pads is masked by cnt=0.
    nc.gpsimd.memset(S4T[:], 0.0)
    nc.gpsimd.memset(DvPs[:], 0.0)
    nc.gpsimd.memset(DvMs[:], 0.0)
    nc.gpsimd.memset(biaseps[:], 1e-6)
    nc.gpsimd.memset(warm[:], 1.0)
    for fn in (AF.Exp, AF.Ln):
        nc.scalar.activation(out=warm[:], in_=warm[:], func=fn)

    nc.sync.dma_start(out=cmt[:], in_=cmd[:])
    return dict(cmt=cmt, selm=selm, S4T=S4T, DvPs=DvPs, DvMs=DvMs,
                biaseps=biaseps)


def emit_rep(nc, pools, cst, x1d, x2d, outd):
    """Emit one rep's input DMAs, odd copies, products and matmuls.

    Returns a closure that emits the rep's stage 2 (psum evacuation, exp,
    cnt-weighted sums, log) — called AFTER the next rep's front has been
    emitted so stage 2 lands behind the next rep's products in the DVE/scalar
    FIFOs instead of head-of-line blocking them.
    """
    AF = mybir.ActivationFunctionType
    OP = mybir.AluOpType
    xp, pp, sp, qp, obp = pools
    cmt, selm, S4T = cst["cmt"], cst["selm"], cst["S4T"]
    DvPs, DvMs, biaseps = cst["DvPs"], cst["DvMs"], cst["biaseps"]

    x1e = xp.tile([128, X1L], _BF16, tag="x1e", name="x1e", bufs=2)
    x2e = xp.tile([128, X2L], _BF16, tag="x2e", name="x2e", bufs=2)
    # odd copies: x1o[k] = x1e[k+1]; single-buffered (range-level deps let
    # the next rep's generation start as soon as this rep's early blocks
    # have been consumed).
    x1o = xp.tile([128, X1L], _BF16, tag="x1o", name="x1o", bufs=1)
    x2o = xp.tile([128, X2L], _BF16, tag="x2o", name="x2o", bufs=1)

    # ---- input DMAs (chunked so consumers unlock progressively) ----
    nc.sync.dma_start(out=x1e[:, 0:2 * PAD], in_=x1d[:, 0:2 * PAD])
    nc.sync.dma_start(out=x1e[:, 2 * PAD:2 * PAD + DMACH],
                      in_=x1d[:, 2 * PAD:2 * PAD + DMACH])
    h1 = PAD + X2SPLIT + PAD
    nc.sync.dma_start(out=x2e[:, 0:h1], in_=x2d[:, 0:h1])
    nc.sync.dma_start(out=x2e[:, h1:X2L], in_=x2d[:, h1:X2L])
    for b in range(1, PIX // DMACH):
        c0 = 2 * PAD + b * DMACH
        c1 = min(c0 + DMACH, X1L) if b < PIX // DMACH - 1 else X1L
        nc.sync.dma_start(out=x1e[:, c0:c1], in_=x1d[:, c0:c1])

    # ---- odd copies on the scalar engine ----
    def gen_odd(dst, src, gs, ge):
        nc.scalar.activation(out=dst[:, gs:ge], in_=src[:, gs + 1:ge + 1],
                             func=AF.Copy)

    gen_odd(x1o, x1e, PAD, PAD + GCH + 127)
    gen_odd(x2o, x2e, PAD, PAD + X2SPLIT + 127)
    gen_odd(x2o, x2e, PAD + X2SPLIT, PAD + X2W + PAD // 2)
    for b in range(1, PIX // GCH):
        gs = PAD + b * GCH
        ge = gs + GCH + 127 if b < PIX // GCH - 1 else 2 * PAD + PIX
        gen_odd(x1o, x1e, gs, ge)

    # ---- stage 1: products (DVE/Pool) + flipped selector matmuls ----
    # full-bank tile (2048B): the rep's single start=True marks exactly this
    # bank's zero-region, so every column's first write overwrites cleanly.
    ptA = qp.tile([128, 512], _F32, tag="psA", name="psA")

    def products(blk, m):
        """Product tile for (block, map): blk 0/1 = x1 halves, 2 = x2."""
        t = pp.tile([128, PPAD], _BF16, tag="prod", name="prod", bufs=3)
        d = DELTAS[m]
        if blk < 2:
            s = PAD + blk * BLK
            ev, od = x1e, x1o
        else:
            s = PAD
            ev, od = x2e, x2o
        in0 = ev[:, s:s + PPAD]
        if d % 2 == 0:
            in1 = ev[:, s + d:s + d + PPAD]
        else:
            in1 = od[:, s + d - 1:s + d - 1 + PPAD]
        split = POOL_SPLIT.get((blk, m), 0)
        cut = PPAD - split
        if cut > 0:
            nc.vector.tensor_tensor(out=t[:, 0:cut], in0=in0[:, 0:cut],
                                    in1=in1[:, 0:cut], op=OP.mult)
        if split > 0:
            nc.gpsimd.tensor_tensor(out=t[:, cut:PPAD], in0=in0[:, cut:PPAD],
                                    in1=in1[:, cut:PPAD], op=OP.mult)
        return t

    first = [True]

    def reduce_x1(t, m, blk):
        """56 per-row ldweights+matmuls: psum[j, CB[m]+row] = channel sums.

        Only the rep's FIRST matmul sets start (marks the whole psum bank
        pending-zero); each column's first writer then overwrites, and the
        x2 fold accumulates on top."""
        for rl in range(RPB):
            # interleaved psum columns: rows r / r+56 sit at 2r / 2r+1 so
            # the x2 fold writes two contiguous columns.
            col = CB[m] + 2 * rl + blk
            nc.tensor.matmul(ptA[:, col:col + 1],
                             t[:, rl * W:rl * W + 128],
                             selm[:, 0:1],
                             start=first[0], stop=False, skip_group_check=True)
            first[0] = False

    def reduce_x2(t, m):
        """56 ldweights + 2-col masked matmuls folding rows r and r+56."""
        for rl in range(RPB):
            col = CB[m] + 2 * rl
            nc.tensor.matmul(ptA[:, col:col + 2], t[:, rl * W:rl * W + 128],
                             selm[:, 1:3],
                             start=False, stop=True, skip_group_check=True)

    for blk in range(3):
        prods = {}
        for m in MAPS:
            prods[m] = products(blk, m)
        for m in MAPS:
            if blk < 2:
                reduce_x1(prods[m], m, blk)
            else:
                reduce_x2(prods[m], m)

    def finish():
        # ---- psum evacuation: one scalar copy into the 4 direct slots ----
        # de-interleave while evacuating: psum col (map, 2a+b) -> slot map,
        # col 1 + a + 56b.
        nc.scalar.activation(
            out=_cap(S4T[0:112], [[114, 4], [1, RPB], [RPB, 2]], 1),
            in_=ptA[0:112, 0:4 * W], func=AF.Copy)
        # slot 4 = S3 at +1 col (reads give S3[i-1]); slot 5 = S1 at -1 col
        # (reads at c-1 give S1[i]).
        nc.sync.dma_start(out=S4T[0:112, 4, 2:114], in_=S4T[0:112, 2, 1:113])
        nc.sync.dma_start(out=S4T[0:112, 5, 0:112], in_=S4T[0:112, 0, 1:113])

        # ---- stage 2: softmax-style assembly (transposed layout) ----
        E4T = sp.tile([128, 5, 114], _F32, tag="E4T", name="E4T", bufs=1)
        nc.scalar.activation(out=E4T[:], in_=S4T[:, 0:5, :], func=AF.Exp)

        outsb = obp.tile([128, 3], _F32, tag="outsb", name="outsb", bufs=2)
        # direct stack: slots 0..4 at col c=i+1, cnt channels 1..5
        cdir = _cap(cmt[:], [[1, W], [W, 5]], 1 * W)
        edir = _cap(E4T[:], [[1, W], [114, 5]], 1)
        sdir = _cap(S4T[:], [[1, W], [114, 5]], 1)
        # +1-shift denom group: E slots (0 @ c, 3 @ c-1): stride 3*114-1
        cpl = _cap(cmt[:], [[1, W], [W, 2]], 6 * W)
        epl = _cap(E4T[:], [[1, W], [341, 2]], 1)
        # -1-shift denom single: E slot 1 @ c-1
        cmi = _cap(cmt[:], [[1, W]], 8 * W)
        emi = _cap(E4T[:], [[1, W]], 114)
        # derived logit 3-stack: S slots (1, 3, 5) @ c-1, cnt channels 9..11
        cder = _cap(cmt[:], [[1, W], [W, 3]], 9 * W)
        sder = _cap(S4T[:], [[1, W], [228, 3]], 114)

        # denominator side
        tdD = sp.tile([128, W, 5], _F32, tag="tdD", name="tdD", bufs=1)
        nc.vector.tensor_tensor(out=tdD[:], in0=cdir, in1=edir, op=OP.mult)
        sumD = sp.tile([128, W], _F32, tag="sumD", name="sumD", bufs=1)
        nc.vector.reduce_sum(out=sumD[:], in_=tdD[:],
                             axis=mybir.AxisListType.X)
        Dv = sp.tile([128, W], _F32, tag="Dv", name="Dv", bufs=1)
        nc.vector.tensor_tensor(out=Dv[:], in0=cmt[:, 0:W], in1=sumD[:],
                                op=OP.add)
        tvP = sp.tile([128, W, 2], _F32, tag="tvP", name="tvP", bufs=1)
        nc.vector.tensor_tensor(out=tvP[:], in0=cpl, in1=epl, op=OP.mult)
        DvP = sp.tile([128, W], _F32, tag="DvP", name="DvP", bufs=1)
        nc.vector.reduce_sum(out=DvP[:], in_=tvP[:],
                             axis=mybir.AxisListType.X)
        DvM = sp.tile([128, W], _F32, tag="DvM", name="DvM", bufs=1)
        nc.vector.tensor_tensor(out=DvM[:], in0=cmi, in1=emi, op=OP.mult)
        # derived contributions move one partition over; realize with DMA
        nc.sync.dma_start(out=DvPs[1:112, :], in_=DvP[0:111, :])
        nc.sync.dma_start(out=DvMs[0:111, :], in_=DvM[1:112, :])

        # logit-sum side (fills the DVE while the shift DMAs fly); the full
        # reduces ride the scalar engine's accum_out to keep DVE cycles low.
        tdS = sp.tile([128, W, 5], _F32, tag="tdS", name="tdS", bufs=1)
        nc.vector.tensor_tensor(out=tdS[:], in0=cdir, in1=sdir, op=OP.mult)
        scD4 = sp.tile([128, W, 5], _F32, tag="scD4", name="scD4", bufs=1)
        nc.scalar.activation(out=scD4[:], in_=tdS[:], func=AF.Copy,
                             accum_out=outsb[:, 1:2])
        tvS = sp.tile([128, W, 3], _F32, tag="tvS", name="tvS", bufs=1)
        nc.vector.tensor_tensor(out=tvS[:], in0=cder, in1=sder, op=OP.mult)
        scD3 = sp.tile([128, W, 3], _F32, tag="scD3", name="scD3", bufs=1)
        nc.scalar.activation(out=scD3[:], in_=tvS[:], func=AF.Copy,
                             accum_out=outsb[:, 2:3])

        nc.vector.tensor_tensor(out=Dv[:], in0=Dv[:], in1=DvPs[:], op=OP.add)
        nc.vector.tensor_tensor(out=Dv[:], in0=Dv[:], in1=DvMs[:], op=OP.add)
        lgd = sp.tile([128, W], _F32, tag="lgd", name="lgd", bufs=1)
        nc.scalar.activation(out=lgd[:], in_=Dv[:], func=AF.Ln,
                             bias=biaseps[:], accum_out=outsb[:, 0:1])
        nc.sync.dma_start(out=outd[:], in_=outsb[:])

    return finish


def build_nc(reps=1):
    """Build + compile the SPMD program. reps>1 unrolls the whole body for
    device-side timing (amortizes the axon tunnel round-trip)."""
    nc = bacc.Bacc("TRN2", target_bir_lowering=False, debug=False,
                   num_devices=CORES)
    x1d = nc.dram_tensor("x1", [128, X1L], _BF16, kind="ExternalInput")
    x2d = nc.dram_tensor("x2", [128, X2L], _BF16, kind="ExternalInput")
    cmd = nc.dram_tensor("cm", [128, NCH_CM * W], _F32, kind="ExternalInput")
    outd = nc.dram_tensor("out", [128, 3], _F32, kind="ExternalOutput")
    with contextlib.ExitStack() as ctx:
        with tile.TileContext(nc) as tc:
            with tc.tile_pool(name="constp", bufs=1) as cp, \
                 tc.tile_pool(name="xp", bufs=2) as xp, \
                 tc.tile_pool(name="prodp", bufs=4) as pp, \
                 tc.tile_pool(name="s2p", bufs=1) as sp, \
                 tc.tile_pool(name="outp", bufs=2) as obp, \
                 tc.tile_pool(name="psump", bufs=2, space="PSUM") as qp:
                cst = emit_consts(nc, cp, cmd)
                pools = (xp, pp, sp, qp, obp)
                fin = None
                for _ in range(reps):
                    nxt = emit_rep(nc, pools, cst, x1d, x2d, outd)
                    if fin is not None:
                        fin()
                    fin = nxt
                fin()
    nc.compile()
    return nc


def pack_inputs(features, labels, directions):
    """Host-side sharding/packing. Returns per-core input dicts."""
    features = np.asarray(features, dtype=np.float32)
    directions = np.asarray(directions)
    labels = np.asarray(labels)

    # normalize + fold the 1/T temperature split across both dot factors:
    # y = x / ||x|| * sqrt(1/T)  =>  <y, y'> = cos(x, x') / T
    nrm = np.sqrt((features * features).sum(axis=1, keepdims=True))
    y = features / np.maximum(nrm, np.float32(1e-12))
    y *= np.float32(np.sqrt(1.0 / TEMP))

    # direction histogram over the batch axis: cnt[k, i, j], k = (di+1)*3+dj+1
    k = (directions[:, 0].astype(np.int64) + 1) * 3 + (directions[:, 1] + 1)
    cnt = np.zeros((9, H, W), np.float32)
    for kk in range(9):
        cnt[kk] = (k == kk).sum(axis=0)

    # If labels are not all identical, fall back to a mask-aware host path
    # (the problem spec fills labels with zeros, so this never triggers).
    uniform_labels = (labels == labels.flat[0]).all()

    e_self = np.exp(np.float32(1.0 / TEMP))
    global _SELF_LS
    _SELF_LS = float(np.float64(cnt[4]).sum() * (1.0 / TEMP))

    # constant-map channels, TRANSPOSED (partition = image column j, free
    # position = image row i).  Direction index k: (di+1)*3 + (dj+1).
    # ch0: self denom; ch1..5: direct stack {(0,1),(1,-1),(1,0),(1,1),(-1,0)}
    # at dest; ch6..7: +1-partition-shift denom group {(0,-1),(-1,-1)} with
    # cnt at dest j=q+1; ch8: -1 group {(-1,1)} with cnt at dest j=q-1;
    # ch9..11: derived logit stack {(-1,1)@q-1, (-1,-1)@q+1, (0,-1)@q+1}.
    cT = cnt.transpose(0, 2, 1)                         # [k, j, i]
    ch = np.zeros((NCH_CM, W, H), np.float32)
    ch[0] = cT[4] * e_self
    for i_, kk in enumerate((5, 6, 7, 8, 1)):
        ch[1 + i_] = cT[kk]
    # cnt_(0,-1)[dest j=q+1] stored at source partition q, etc.
    ch[6] = np.roll(cT[3], -1, axis=0); ch[6][W - 1] = 0
    ch[7] = np.roll(cT[0], -1, axis=0); ch[7][W - 1] = 0
    ch[8] = np.roll(cT[2], 1, axis=0); ch[8][0] = 0
    ch[9] = ch[8]
    ch[10] = ch[7]
    ch[11] = ch[6]
    chT = ch.transpose(1, 0, 2)                          # (j, ch, i)
    cm = np.zeros((128, NCH_CM, H), np.float32)
    cm[0:W] = chT
    cm = np.ascontiguousarray(cm.reshape(128, NCH_CM * H))

    in_maps = []
    for core in range(CORES):
        xb = y[core].astype(bfloat16)                    # (C, H, W)
        x1 = np.zeros((128, X1L), bfloat16)
        x1[:, PAD:PAD + PIX] = xb[:128].reshape(128, PIX)
        hi = xb[128:]                                    # (64, H, W)
        x2 = np.zeros((128, X2L), bfloat16)
        x2[0:64, PAD:PAD + X2W] = hi[:, 0:X2R].reshape(64, X2W)
        lower_rows = np.clip(np.arange(56, 56 + X2R), 0, H - 1)
        x2[64:128, PAD:PAD + X2W] = hi[:, lower_rows].reshape(64, X2W)
        in_maps.append({"x1": x1, "x2": x2, "cm": cm})
    return in_maps, uniform_labels


_SELF_LS = 0.0


def unpack_loss(results):
    """Combine per-core [128, 3] partials into the scalar loss.

    cols: 0 = sum_i log(denom), 1 = direct + (-1,0) logit sums, 2 = derived
    logit sums (at source columns).  The self-map logit sum (cnt4/T, a host
    constant) is added here.
    """
    lg_sum = 0.0
    ls_sum = 0.0
    for core in range(CORES):
        o = np.asarray(results[core]["out"], np.float64)
        lg_sum += o[0:W, 0].sum()
        ls_sum += o[0:W, 1].sum() + o[0:W, 2].sum() + _SELF_LS
    loss = lg_sum / (N * H * W) - ls_sum / (N * N * H * W)
    return np.float32(loss)


_NC_CACHE = {}


def _get_nc(reps=1):
    if reps not in _NC_CACHE:
        _NC_CACHE[reps] = build_nc(reps)
    return _NC_CACHE[reps]


def _host_reference_loss(features, labels, directions):
    """Mask-aware fallback (numpy, fp32) for non-uniform labels."""
    f = np.asarray(features, np.float32)
    nrm = np.sqrt((f * f).sum(axis=1, keepdims=True))
    fn = f / np.maximum(nrm, 1e-12)
    ii = np.arange(H)[None, :, None]
    jj = np.arange(W)[None, None, :]
    ni = ii + directions[:, 0]
    nj = jj + directions[:, 1]
    gathered = fn[:, :, ni, nj]                 # (N, C, M, H, W)
    logits = np.einsum('ncij,ncmij->nmij', fn, gathered) / TEMP
    lab = np.asarray(labels)
    labels_g = lab[:, ni, nj]
    mask = (lab[None, :, :, :] == labels_g).astype(np.float32)
    exp_l = np.exp(logits) * mask
    denom = exp_l.sum(axis=1, keepdims=True)
    return np.float32((-np.log(exp_l / (denom + 1e-6))).mean())


def kernel(features, labels, directions):
    in_maps, uniform = pack_inputs(features, labels, directions)
    if not uniform:
        return _host_reference_loss(features, labels, directions)
    nc = _get_nc()
    res = run_bass_kernel_spmd(nc, in_maps, core_ids=list(range(CORES)))
    return unpack_loss(res.results)
